# revision 1
# baseline (speedup 1.0000x reference)
"""Trainium2 Bass kernel for nn_LogicGatedSpikingSelfAttention.

Sharding: channel/head-parallel over 8 cores. Each core owns 128 output
channels = 2 heads for the q/k/v branches (BN stats fully local, since
stats are per-channel over all tokens), runs attention for its 2 heads
over all 4 batches locally, and computes a 128-output-channel slice of
the projection. One AllGather moves the binary attention spikes (+ per-
head energies for the logic gate) between the attention and projection
stages; the gate is folded into the projection weights after the gather
(exact: gate is {0,1}).

Numerics: all big matmuls in bf16. The attention is exact in integers
(spikes are {0,1}: counts accumulate exactly in fp32 PSUM, and the
attn-LIF threshold reduces to an integer compare S >= 2^0.75). The LIF
forward pass is a pure Heaviside, so each branch reduces to
Y >= m + (2-beta)/gamma * sqrt(var+eps) with per-channel scalars.
"""
import numpy as np
import ml_dtypes

import concourse.bass as bass
import concourse.bacc as bacc
import concourse.tile as tile
from concourse import mybir
from concourse.bass_utils import run_bass_kernel_spmd

NCORES = 8
B, NSEQ, D, H = 4, 1024, 1024, 16
HD = D // H            # 64 head dim
CH = D // NCORES       # 128 channels per core
TOK = B * NSEQ         # 4096 tokens
KT = D // 128          # 8 contraction tiles
EPS = 1e-5
S_TH = float(2.0 ** 0.75)   # x_attn >= 1  <=>  S >= hd**0.125 = 2^0.75
SPIKE_N = 128 * TOK         # flat payload: spikes then 8 energy slots
PAYLEN = SPIKE_N + 8
F32 = mybir.dt.float32
BF16 = mybir.dt.bfloat16
BF = ml_dtypes.bfloat16

_CACHE = {}


def _build():
    nc = bacc.Bacc("TRN2", target_bir_lowering=False, debug=False,
                   num_devices=NCORES)
    inp = {}
    def din(name, shape, dt=BF16):
        inp[name] = nc.dram_tensor(name, shape, dt, kind="ExternalInput")
        return inp[name]

    din("xT",  [128, KT * TOK])          # host pre-tiled: [p, (t n)]
    din("wq",  [128, KT * CH]); din("wk", [128, KT * CH])
    din("wv",  [128, KT * CH]); din("wp", [128, KT * CH])
    for nm in ("tq", "tk", "tv", "tp", "bq", "bk", "bv", "bp"):
        din(nm, [CH, 1], F32)
    din("wgr", [H, H], F32)              # lhsT: [h, h'] = sum_r Wg[h', h+16r]/1024
    din("bgr", [H, 1], F32)
    din("i2e", [CH, 2], F32)             # [p, j] = (p//64==j)
    din("i16", [H, KT * 128], F32)       # [h, (t m)] = (t*128+m)//64 == h
    din("idn", [128, 128])               # identity for PE transpose
    outT = nc.dram_tensor("outT", [CH, TOK], BF16, kind="ExternalOutput")

    with tile.TileContext(nc) as tc:
        with tc.tile_pool(name="consts", bufs=1) as consts, \
             tc.tile_pool(name="spikes", bufs=1) as spk_pool, \
             tc.tile_pool(name="dram", bufs=1, space="DRAM") as dram:
            _body(tc, inp, outT, consts, spk_pool, dram)
    nc.compile()
    return nc


def _body(tc, inp, outT, consts, spk_pool, dram):
    nc = tc.nc
    V, SC, GP, TE = nc.vector, nc.scalar, nc.gpsimd, nc.tensor
    AF = mybir.ActivationFunctionType
    OP = mybir.AluOpType
    DENG = [nc.sync, nc.scalar, nc.gpsimd]

    # ---- constants / weights to SBUF (all host-contiguous) ----
    w_sb = {}
    for i, nm in enumerate(("wq", "wk", "wv", "wp")):
        t = consts.tile([128, KT, CH], BF16, name=f"{nm}_sb")
        DENG[i % 3].dma_start(
            t[:], inp[nm].ap().rearrange("p (t m) -> p t m", t=KT))
        w_sb[nm] = t
    small = {}
    for nm in ("tq", "tk", "tv", "tp", "bq", "bk", "bv", "bp", "bgr"):
        t = consts.tile([inp[nm].shape[0], 1], F32, name=f"{nm}_sb")
        nc.sync.dma_start(t[:], inp[nm].ap())
        small[nm] = t
    wgr_sb = consts.tile([H, H], F32)
    nc.sync.dma_start(wgr_sb[:], inp["wgr"].ap())
    i2e_sb = consts.tile([CH, 2], F32)
    nc.sync.dma_start(i2e_sb[:], inp["i2e"].ap())
    i16_sb = consts.tile([H, KT, 128], F32)
    nc.sync.dma_start(i16_sb[:],
                      inp["i16"].ap().rearrange("h (t m) -> h t m", t=KT))
    idn_sb = consts.tile([128, 128], BF16)
    nc.scalar.dma_start(idn_sb[:], inp["idn"].ap())
    eps_sb = consts.tile([128, 1], F32)
    V.memset(eps_sb[:], EPS)

    # ---- persistent spike tensors ----
    spA = {nm: spk_pool.tile([128, TOK], BF16, name=f"sp{nm}A")
           for nm in ("q", "k", "v")}
    sp2 = {nm: spk_pool.tile([HD, 2, TOK], BF16, name=f"sp2{nm}")
           for nm in ("q", "k")}
    vnat = spk_pool.tile([128, 32, 128], BF16)          # [tok, b*8+mt, ch]
    payload = spk_pool.tile([HD, 2, TOK], BF16)         # [d, h, tok] spikes

    # ================= branches (q, k, v) =================
    with tc.tile_pool(name="xts_p", bufs=1) as xts_p, \
         tc.tile_pool(name="ybig", bufs=2) as ybig, \
         tc.tile_pool(name="stps", bufs=2) as stp:
        xts = xts_p.tile([128, KT, TOK], BF16)
        nc.gpsimd.dma_start(
            xts[:], inp["xT"].ap().rearrange("p (t n) -> p t n", t=KT))

        for nm in ("q", "k", "v"):
            Y = ybig.tile([128, TOK], F32, tag="Y")
            # weight-stationary: kt outer, 8 PSUM banks accumulate
            with tc.tile_pool(name=f"brps_{nm}", bufs=1, space="PSUM") as brps:
                ps = [brps.tile([128, 512], F32, name=f"ps{nm}{i}")
                      for i in range(8)]
                for kt in range(KT):
                    for nck in range(8):
                        TE.matmul(ps[nck][:], w_sb["w" + nm][:, kt, :],
                                  xts[:, kt, nck * 512:(nck + 1) * 512],
                                  start=(kt == 0), stop=(kt == KT - 1))
                for nck in range(8):
                    if nck % 2:
                        V.tensor_scalar(Y[:, nck * 512:(nck + 1) * 512],
                                        ps[nck][:], small["b" + nm][:],
                                        None, OP.add)
                    else:
                        SC.activation(Y[:, nck * 512:(nck + 1) * 512],
                                      ps[nck][:], AF.Identity,
                                      bias=small["b" + nm][:])
            stats = stp.tile([128, 8, 6], F32, tag="stats")
            for i in range(8):
                V.bn_stats(stats[:, i, :], Y[:, i * 512:(i + 1) * 512])
            mv = stp.tile([128, 2], F32, tag="mv")
            V.bn_aggr(mv[:], stats[:])
            std = stp.tile([128, 1], F32, tag="std")
            SC.activation(std[:], mv[:, 1:2], AF.Sqrt, bias=eps_sb[:])
            thr = stp.tile([128, 1], F32, tag="thr")
            V.tensor_tensor(thr[:], std[:], small["t" + nm][:], OP.mult)
            V.tensor_tensor(thr[:], thr[:], mv[:, 0:1], OP.add)
            V.tensor_scalar(spA[nm][:], Y[:], thr[:], None, OP.is_ge)

        # head-split q, k for attention operand layout (base partition 0)
        for i, nm in enumerate(("q", "k")):
            for h in range(2):
                DENG[(i * 2 + h) % 3].dma_start(
                    sp2[nm][:, h, :], spA[nm][h * HD:(h + 1) * HD, :])

        # v -> natural (token, channel) layout via PE transpose
        with tc.tile_pool(name="tps", bufs=4, space="PSUM") as tps:
            for i in range(32):
                vt = tps.tile([128, 128], BF16, tag="vt")
                TE.transpose(vt[:], spA["v"][:, i * 128:(i + 1) * 128], idn_sb[:])
                if i % 2 == 0:
                    V.tensor_copy(vnat[:, i, :], vt[:])
                else:
                    SC.activation(vnat[:, i, :], vt[:], AF.Copy)

    # ================= energy =================
    e_sb = spk_pool.tile([2, B], BF16)
    with tc.tile_pool(name="enps", bufs=1, space="PSUM") as enps, \
         tc.tile_pool(name="entmp", bufs=1) as entmp:
        prod = entmp.tile([128, TOK], BF16)
        V.tensor_tensor(prod[:], spA["q"][:], spA["k"][:], OP.mult)
        ech = entmp.tile([128, B], F32)
        V.reduce_sum(ech[:], prod[:].rearrange("p (b n) -> p b n", b=B),
                     axis=mybir.AxisListType.X)
        e_ps = enps.tile([2, B], F32)
        TE.matmul(e_ps[:], i2e_sb[:], ech[:], start=True, stop=True)
        V.tensor_copy(e_sb[:], e_ps[:])

    # ================= attention =================
    with tc.tile_pool(name="cps", bufs=3, space="PSUM") as cps, \
         tc.tile_pool(name="sps", bufs=4, space="PSUM") as sps, \
         tc.tile_pool(name="csb", bufs=4) as csb:
        for b in range(B):
            for h in range(2):
                s_ps = [sps.tile([HD, 512], F32, tag="sps", name=f"s_ps{b}{h}{i}")
                        for i in range(2)]
                for mt in range(8):
                    m0 = b * NSEQ + mt * 128
                    for ncn in range(2):
                        n0 = b * NSEQ + ncn * 512
                        c_ps = cps.tile([128, 512], F32, tag="cps")
                        TE.matmul(c_ps[:], sp2["k"][:, h, m0:m0 + 128],
                                  sp2["q"][:, h, n0:n0 + 512],
                                  start=True, stop=True)
                        c_sb = csb.tile([128, 512], BF16, tag="csb")
                        if (mt * 2 + ncn) % 4 == 3:
                            SC.activation(c_sb[:], c_ps[:], AF.Copy)
                        else:
                            V.tensor_copy(c_sb[:], c_ps[:])
                        TE.matmul(s_ps[ncn][:],
                                  vnat[:, b * 8 + mt, h * HD:(h + 1) * HD],
                                  c_sb[:], start=(mt == 0), stop=(mt == 7))
                for ncn in range(2):
                    n0 = b * NSEQ + ncn * 512
                    V.tensor_scalar(payload[:, h, n0:n0 + 512], s_ps[ncn][:],
                                    S_TH, None, OP.is_ge)

    # ================= AllGather (flat, contiguous) =================
    pay_d = dram.tile([PAYLEN], BF16)
    gath_d = dram.tile([NCORES, PAYLEN], BF16, addr_space="Shared")
    for h in range(2):
        DENG[h].dma_start(
            pay_d[h * HD * TOK:(h + 1) * HD * TOK].rearrange(
                "(p n) -> p n", p=HD),
            payload[:, h, :])
    nc.sync.dma_start(
        pay_d[SPIKE_N:SPIKE_N + 8].rearrange("(p w) -> p w", p=2), e_sb[:])
    GP.collective_compute("AllGather", OP.bypass,
                          ins=[pay_d.opt()], outs=[gath_d.opt()],
                          replica_groups=[list(range(NCORES))])

    # ================= gate -> gated proj weights =================
    with tc.tile_pool(name="gtmp", bufs=1) as gtmp, \
         tc.tile_pool(name="post", bufs=1) as post, \
         tc.tile_pool(name="pstat", bufs=1) as pstat:
        with tc.tile_pool(name="gtps", bufs=2, space="PSUM") as gtps:
            eg_bf = gtmp.tile([H, B], BF16)
            nc.sync.dma_start(
                eg_bf[:],
                gath_d[:, SPIKE_N:SPIKE_N + 8].rearrange(
                    "c (p w) -> c p w", p=2))
            eg = gtmp.tile([H, B], F32)
            V.tensor_copy(eg[:], eg_bf[:])
            g_ps = gtps.tile([H, B], F32, tag="gps")
            TE.matmul(g_ps[:], wgr_sb[:], eg[:], start=True, stop=True)
            gate = gtmp.tile([H, B], F32)
            V.tensor_scalar(gate[:], g_ps[:], small["bgr"][:], 0.5,
                            OP.add, OP.is_ge)
            gv = gtmp.tile([128, KT, B], F32)
            for t in range(KT):
                gv_ps = gtps.tile([128, B], F32, tag="gvps")
                TE.matmul(gv_ps[:], i16_sb[:, t, :], gate[:],
                          start=True, stop=True)
                V.tensor_copy(gv[:, t, :], gv_ps[:])
            wpg = post.tile([128, KT, B, 128], BF16)
            for t in range(KT):
                for b in range(B):
                    if (t * B + b) % 2:
                        V.tensor_scalar(wpg[:, t, b, :], w_sb["wp"][:, t, :],
                                        gv[:, t, b:b + 1], None, OP.mult)
                    else:
                        SC.activation(wpg[:, t, b, :], w_sb["wp"][:, t, :],
                                      AF.Identity, scale=gv[:, t, b:b + 1])

        # ================= projection =================
        rhs = [post.tile([128, TOK], BF16, name=f"rhs{t}") for t in range(KT)]
        for t in range(KT):
            DENG[t % 3].dma_start(
                rhs[t][:],
                gath_d[t, 0:SPIKE_N].rearrange("(p n) -> p n", p=128))
        Yp = post.tile([128, TOK], F32)
        with tc.tile_pool(name="ppps", bufs=1, space="PSUM") as ppps:
            pp = [ppps.tile([128, 512], F32, name=f"pp{i}") for i in range(8)]
            for t in range(KT):
                for b in range(B):
                    for ncn in range(2):
                        n0 = b * NSEQ + ncn * 512
                        TE.matmul(pp[b * 2 + ncn][:], wpg[:, t, b, :],
                                  rhs[t][:, n0:n0 + 512],
                                  start=(t == 0), stop=(t == KT - 1))
            for i in range(8):
                if i % 2:
                    V.tensor_scalar(Yp[:, i * 512:(i + 1) * 512], pp[i][:],
                                    small["bp"][:], None, OP.add)
                else:
                    SC.activation(Yp[:, i * 512:(i + 1) * 512], pp[i][:],
                                  AF.Identity, bias=small["bp"][:])
        stats = pstat.tile([128, 8, 6], F32)
        for i in range(8):
            V.bn_stats(stats[:, i, :], Yp[:, i * 512:(i + 1) * 512])
        mv = pstat.tile([128, 2], F32)
        V.bn_aggr(mv[:], stats[:])
        std = pstat.tile([128, 1], F32)
        SC.activation(std[:], mv[:, 1:2], AF.Sqrt, bias=eps_sb[:])
        thr = pstat.tile([128, 1], F32)
        V.tensor_tensor(thr[:], std[:], small["tp"][:], OP.mult)
        V.tensor_tensor(thr[:], thr[:], mv[:, 0:1], OP.add)
        osb = pstat.tile([128, TOK], BF16)
        V.tensor_scalar(osb[:], Yp[:], thr[:], None, OP.is_ge)
        nc.sync.dma_start(outT.ap(), osb[:])


def _tile_rows(a):
    # (8*128, N) -> (128, 8*N) so the SBUF [p, (t n)] load is contiguous
    n = a.shape[1]
    return np.ascontiguousarray(
        a.reshape(KT, 128, n).transpose(1, 0, 2).reshape(128, KT * n))


def _prep_inputs(inputs):
    x = np.asarray(inputs["x"], np.float32)
    xT = _tile_rows(x.reshape(TOK, D).T.astype(BF))
    Wg = np.asarray(inputs["Wg"], np.float64)
    wgr = (Wg.reshape(H, HD, H).sum(axis=1).T / 1024.0).astype(np.float32)
    wgr = np.ascontiguousarray(wgr)                     # [h, h']
    bgr = np.asarray(inputs["bg"], np.float32).reshape(H, 1)
    i2e = np.zeros((CH, 2), np.float32)
    i2e[0:HD, 0] = 1.0
    i2e[HD:CH, 1] = 1.0
    i16 = np.zeros((H, D), np.float32)
    for h in range(H):
        i16[h, h * HD:(h + 1) * HD] = 1.0
    i16 = np.ascontiguousarray(
        i16.reshape(H, KT, 128).reshape(H, KT * 128))
    idn = np.eye(128, dtype=BF)
    in_maps = []
    for c in range(NCORES):
        sl = slice(CH * c, CH * c + CH)
        m = {"xT": xT, "wgr": wgr, "bgr": bgr, "i2e": i2e, "i16": i16,
             "idn": idn}
        for nm in ("q", "k", "v", "p"):
            W = np.asarray(inputs[f"W{nm}"], np.float32)
            m["w" + nm] = _tile_rows(W[sl, :].T.astype(BF))
            g = np.asarray(inputs[f"g{nm}"], np.float32)[sl]
            be = np.asarray(inputs[f"beta{nm}"], np.float32)[sl]
            m["t" + nm] = ((2.0 - be) / g).reshape(CH, 1).astype(np.float32)
            m["b" + nm] = np.asarray(
                inputs[f"b{nm}"], np.float32)[sl].reshape(CH, 1)
        in_maps.append(m)
    return in_maps


def _run(inputs, trace=False):
    if "nc" not in _CACHE:
        _CACHE["nc"] = _build()
    nc = _CACHE["nc"]
    in_maps = _prep_inputs(inputs)
    res = run_bass_kernel_spmd(nc, in_maps, core_ids=list(range(NCORES)),
                               trace=trace)
    out = np.empty((TOK, D), np.float32)
    for c in range(NCORES):
        out[:, CH * c:CH * c + CH] = res.results[c]["outT"].astype(np.float32).T
    return out.reshape(B, NSEQ, D), res


def kernel(**inputs) -> np.ndarray:
    out, _ = _run(inputs, trace=False)
    return out



# revision 4
# speedup vs baseline: 1.6698x; 1.6698x over previous
"""Trainium2 Bass kernel for nn_LogicGatedSpikingSelfAttention.

Sharding: channel/head-parallel over 8 cores. Each core owns 128 output
channels = 2 heads for the q/k/v branches (BN stats fully local, since
stats are per-channel over all tokens), runs attention for its 2 heads
over all 4 batches locally, and computes a 128-output-channel slice of
the projection. One AllGather moves the binary attention spikes (+ per-
head energies for the logic gate) between the attention and projection
stages; the gate is folded into the projection weights after the gather
(exact: gate is {0,1}).

Attention uses associativity — there is no softmax, so
x_attn = scale*gate * q @ (k^T @ v). k^T@v is a 64x64 integer count
matrix per (batch, head) (exact in fp16, counts <= 1024), and
S = (k^T v)^T q gives channel-major integer scores identical to the
naive q@k^T@v order. The attn-LIF threshold reduces to S >= 2^0.75.
k/v spikes are transposed to token-major via the DMA XBAR (off the PE).
Per-head small matmuls are packed into PE quadrants via tile_position.
The spike payload crosses cores as fp8e4 ({0,1} exact), energies ride
along as bitcast f32 bytes.
"""
import numpy as np
import ml_dtypes

import concourse.bass as bass
import concourse.bacc as bacc
import concourse.tile as tile
from concourse import mybir
from concourse.bass_utils import run_bass_kernel_spmd

NCORES = 8
B, NSEQ, D, H = 4, 1024, 1024, 16
HD = D // H            # 64 head dim
CH = D // NCORES       # 128 channels per core
TOK = B * NSEQ         # 4096 tokens
KT = D // 128          # 8 contraction tiles
NBLK = TOK // 128      # 32 token blocks of 128
EPS = 1e-5
S_TH = float(2.0 ** 0.75)   # x_attn >= 1  <=>  S >= hd**0.125 = 2^0.75
SPIKE_N = 128 * TOK         # flat fp8 payload: spikes then 32B f32 energies
PAYLEN = SPIKE_N + 32
F32 = mybir.dt.float32
BF16 = mybir.dt.bfloat16
FP16 = mybir.dt.float16
FP8 = mybir.dt.float8e4
BF = ml_dtypes.bfloat16

_CACHE = {}


def _build():
    nc = bacc.Bacc("TRN2", target_bir_lowering=False, debug=False,
                   num_devices=NCORES)
    inp = {}
    def din(name, shape, dt=BF16):
        inp[name] = nc.dram_tensor(name, shape, dt, kind="ExternalInput")
        return inp[name]

    din("xT",  [128, KT * TOK])          # host pre-tiled: [p, (t n)]
    din("wq",  [128, KT * CH]); din("wk", [128, KT * CH])
    din("wv",  [128, KT * CH]); din("wp", [128, KT * CH])
    for nm in ("tq", "tk", "tv", "tp", "bq", "bk", "bv", "bp"):
        din(nm, [CH, 1], F32)
    din("wgr", [H, H], F32)              # lhsT: [h, h'] = sum_r Wg[h', h+16r]/1024
    din("bgr", [H, 1], F32)
    din("i2e", [CH, 2], F32)             # [p, j] = (p//64==j)
    din("i16", [H, KT * 128], F32)       # [h, (t m)] = (t*128+m)//64 == h
    outT = nc.dram_tensor("outT", [CH, TOK], BF16, kind="ExternalOutput")

    with tile.TileContext(nc) as tc:
        with tc.tile_pool(name="consts", bufs=1) as consts, \
             tc.tile_pool(name="spikes", bufs=1) as spk_pool, \
             tc.tile_pool(name="dram", bufs=1, space="DRAM") as dram:
            _body(tc, inp, outT, consts, spk_pool, dram)
    nc.compile()
    return nc


def _body(tc, inp, outT, consts, spk_pool, dram):
    nc = tc.nc
    V, SC, GP, TE = nc.vector, nc.scalar, nc.gpsimd, nc.tensor
    AF = mybir.ActivationFunctionType
    OP = mybir.AluOpType
    DENG = [nc.sync, nc.scalar, nc.gpsimd]

    # ---- constants / weights to SBUF (all host-contiguous) ----
    w_sb = {}
    for i, nm in enumerate(("wq", "wk", "wv", "wp")):
        t = consts.tile([128, KT, CH], BF16, name=f"{nm}_sb")
        DENG[i % 3].dma_start(
            t[:], inp[nm].ap().rearrange("p (t m) -> p t m", t=KT))
        w_sb[nm] = t
    small = {}
    for nm in ("tq", "tk", "tv", "tp", "bq", "bk", "bv", "bp", "bgr"):
        t = consts.tile([inp[nm].shape[0], 1], F32, name=f"{nm}_sb")
        nc.sync.dma_start(t[:], inp[nm].ap())
        small[nm] = t
    wgr_sb = consts.tile([H, H], F32)
    nc.sync.dma_start(wgr_sb[:], inp["wgr"].ap())
    i2e_sb = consts.tile([CH, 2], F32)
    nc.sync.dma_start(i2e_sb[:], inp["i2e"].ap())
    i16_sb = consts.tile([H, KT, 128], F32)
    nc.sync.dma_start(i16_sb[:],
                      inp["i16"].ap().rearrange("h (t m) -> h t m", t=KT))
    eps_sb = consts.tile([128, 1], F32)
    V.memset(eps_sb[:], EPS)

    # ---- persistent spike tensors ----
    spA = {nm: spk_pool.tile([128, TOK], FP16, name=f"sp{nm}A")
           for nm in ("q", "k", "v")}
    ktok = spk_pool.tile([128, NBLK, 128], FP16)   # [tok, blk, ch]
    vtok = spk_pool.tile([128, NBLK, 128], FP16)
    payload = spk_pool.tile([128, TOK], FP8)       # [64h+d, tok] spikes
    e_sb = spk_pool.tile([2, B], F32)

    # ================= branches (q, k, v) =================
    with tc.tile_pool(name="xts_p", bufs=1) as xts_p, \
         tc.tile_pool(name="ybig", bufs=2) as ybig, \
         tc.tile_pool(name="stps", bufs=2) as stp:
        xts = xts_p.tile([128, KT, TOK], BF16)
        xre = inp["xT"].ap().rearrange("p (t n) -> p t n", t=KT)
        for kt in range(KT):
            DENG[kt % 3].dma_start(xts[:, kt, :], xre[:, kt, :])

        for nm in ("q", "k", "v"):
            Y = ybig.tile([128, TOK], F32, tag="Y")
            # weight-stationary: kt outer, 8 PSUM banks accumulate
            with tc.tile_pool(name=f"brps_{nm}", bufs=1, space="PSUM") as brps:
                ps = [brps.tile([128, 512], F32, name=f"ps{nm}{i}")
                      for i in range(8)]
                for kt in range(KT):
                    for nck in range(8):
                        TE.matmul(ps[nck][:], w_sb["w" + nm][:, kt, :],
                                  xts[:, kt, nck * 512:(nck + 1) * 512],
                                  start=(kt == 0), stop=(kt == KT - 1))
                for nck in range(8):
                    if nck % 2:
                        V.tensor_scalar(Y[:, nck * 512:(nck + 1) * 512],
                                        ps[nck][:], small["b" + nm][:],
                                        None, OP.add)
                    else:
                        SC.activation(Y[:, nck * 512:(nck + 1) * 512],
                                      ps[nck][:], AF.Identity,
                                      bias=small["b" + nm][:])
            stats = stp.tile([128, 8, 6], F32, tag="stats")
            for i in range(8):
                V.bn_stats(stats[:, i, :], Y[:, i * 512:(i + 1) * 512])
            mv = stp.tile([128, 2], F32, tag="mv")
            V.bn_aggr(mv[:], stats[:])
            std = stp.tile([128, 1], F32, tag="std")
            SC.activation(std[:], mv[:, 1:2], AF.Sqrt, bias=eps_sb[:])
            thr = stp.tile([128, 1], F32, tag="thr")
            V.tensor_tensor(thr[:], std[:], small["t" + nm][:], OP.mult)
            V.tensor_tensor(thr[:], thr[:], mv[:, 0:1], OP.add)
            V.tensor_scalar(spA[nm][:], Y[:], thr[:], None, OP.is_ge)

            # token-major spikes for k/v via DMA XBAR (off the PE)
            if nm == "k":
                nc.sync.dma_start_transpose(ktok[:], spA["k"][:])
            elif nm == "v":
                nc.scalar.dma_start_transpose(vtok[:], spA["v"][:])

        # energy elementwise part on gpsimd (overlaps v branch)
        prod = spk_pool.tile([128, TOK], FP16)
        GP.tensor_tensor(prod[:], spA["q"][:], spA["k"][:], OP.mult)
        ech = spk_pool.tile([128, B], F32)
        V.reduce_sum(ech[:], prod[:].rearrange("p (b n) -> p b n", b=B),
                     axis=mybir.AxisListType.X)

    # ================= energy head-sum + attention =================
    with tc.tile_pool(name="atps", bufs=1, space="PSUM") as atps, \
         tc.tile_pool(name="s2ps", bufs=2, space="PSUM") as s2ps, \
         tc.tile_pool(name="kvsb", bufs=1) as kvsb:
        e_ps = atps.tile([2, B], F32, name="eps")
        TE.matmul(e_ps[:], i2e_sb[:], ech[:], start=True, stop=True)
        V.tensor_copy(e_sb[:], e_ps[:])

        # KV[b] = k_tok^T @ v_tok per head, heads packed in PE columns
        kv_ps = [atps.tile([128, HD], F32, name=f"kvps{b}") for b in range(B)]
        kv = kvsb.tile([128, B, HD], FP16)
        for b in range(B):
            for mt in range(8):
                blk = b * 8 + mt
                TE.matmul(kv_ps[b][0:HD, :], ktok[:, blk, 0:HD],
                          vtok[:, blk, 0:HD],
                          start=(mt == 0), stop=(mt == 7),
                          tile_position=(0, 0))
                TE.matmul(kv_ps[b][HD:128, :], ktok[:, blk, HD:128],
                          vtok[:, blk, HD:128],
                          start=(mt == 0), stop=(mt == 7),
                          tile_position=(0, HD))
            if b % 2:
                V.tensor_copy(kv[:, b, :], kv_ps[b][:])
            else:
                SC.activation(kv[:, b, :], kv_ps[b][:], AF.Copy)

        # S^T = KV^T @ q  (channel-major scores), heads packed in quadrants
        for b in range(B):
            for ncn in range(2):
                n0 = b * NSEQ + ncn * 512
                s2 = s2ps.tile([128, 512], F32, tag="s2")
                TE.matmul(s2[0:HD, :], kv[0:HD, b, :],
                          spA["q"][0:HD, n0:n0 + 512],
                          start=True, stop=True, tile_position=(0, 0))
                TE.matmul(s2[HD:128, :], kv[HD:128, b, :],
                          spA["q"][HD:128, n0:n0 + 512],
                          start=True, stop=True, tile_position=(HD, HD))
                V.tensor_scalar(payload[:, n0:n0 + 512], s2[:], S_TH,
                                None, OP.is_ge)

    # ================= AllGather (flat fp8, contiguous) =================
    pay_d = dram.tile([PAYLEN], FP8)
    gath_d = dram.tile([NCORES, PAYLEN], FP8, addr_space="Shared")
    nc.sync.dma_start(
        pay_d[0:SPIKE_N].rearrange("(p n) -> p n", p=128), payload[:])
    nc.scalar.dma_start(
        pay_d[SPIKE_N:PAYLEN].rearrange("(p w) -> p w", p=2),
        e_sb[:].bitcast(FP8))
    GP.collective_compute("AllGather", OP.bypass,
                          ins=[pay_d.opt()], outs=[gath_d.opt()],
                          replica_groups=[list(range(NCORES))])

    # ================= gate -> gated proj weights =================
    with tc.tile_pool(name="gtmp", bufs=1) as gtmp, \
         tc.tile_pool(name="post", bufs=1) as post, \
         tc.tile_pool(name="rhsp", bufs=3) as rhsp, \
         tc.tile_pool(name="pstat", bufs=1) as pstat:
        with tc.tile_pool(name="gtps", bufs=2, space="PSUM") as gtps:
            eg_bytes = gtmp.tile([H, 16], FP8)
            for c in range(NCORES):
                DENG[c % 3].dma_start(
                    eg_bytes[2 * c:2 * c + 2, :],
                    gath_d[c, SPIKE_N:PAYLEN].rearrange("(p w) -> p w", p=2))
            g_ps = gtps.tile([H, B], F32, tag="gps")
            TE.matmul(g_ps[:], wgr_sb[:], eg_bytes[:].bitcast(F32),
                      start=True, stop=True)
            gate = gtmp.tile([H, B], F32)
            V.tensor_scalar(gate[:], g_ps[:], small["bgr"][:], 0.5,
                            OP.add, OP.is_ge)
            gv = gtmp.tile([128, KT, B], F32)
            for t in range(KT):
                gv_ps = gtps.tile([128, B], F32, tag="gvps")
                TE.matmul(gv_ps[:], i16_sb[:, t, :], gate[:],
                          start=True, stop=True)
                V.tensor_copy(gv[:, t, :], gv_ps[:])
            wpg = post.tile([128, KT, B, 128], BF16)
            for t in range(KT):
                for b in range(B):
                    if (t * B + b) % 2:
                        V.tensor_scalar(wpg[:, t, b, :], w_sb["wp"][:, t, :],
                                        gv[:, t, b:b + 1], None, OP.mult)
                    else:
                        SC.activation(wpg[:, t, b, :], w_sb["wp"][:, t, :],
                                      AF.Identity, scale=gv[:, t, b:b + 1])

        # ================= projection (fp8 rhs, bf16 weights) ==========
        Yp = post.tile([128, TOK], F32)
        with tc.tile_pool(name="ppps", bufs=1, space="PSUM") as ppps:
            pp = [ppps.tile([128, 512], F32, name=f"pp{i}") for i in range(8)]
            rhs = []
            for t in range(KT):
                r = rhsp.tile([128, TOK], FP8, tag="rhs")
                DENG[t % 3].dma_start(
                    r[:],
                    gath_d[t, 0:SPIKE_N].rearrange("(p n) -> p n", p=128))
                rhs.append(r)
            for t in range(KT):
                for b in range(B):
                    for ncn in range(2):
                        n0 = b * NSEQ + ncn * 512
                        TE.matmul(pp[b * 2 + ncn][:], wpg[:, t, b, :],
                                  rhs[t][:, n0:n0 + 512],
                                  start=(t == 0), stop=(t == KT - 1))
            for i in range(8):
                if i % 2:
                    V.tensor_scalar(Yp[:, i * 512:(i + 1) * 512], pp[i][:],
                                    small["bp"][:], None, OP.add)
                else:
                    SC.activation(Yp[:, i * 512:(i + 1) * 512], pp[i][:],
                                  AF.Identity, bias=small["bp"][:])
        stats = pstat.tile([128, 8, 6], F32)
        for i in range(8):
            V.bn_stats(stats[:, i, :], Yp[:, i * 512:(i + 1) * 512])
        mv = pstat.tile([128, 2], F32)
        V.bn_aggr(mv[:], stats[:])
        std = pstat.tile([128, 1], F32)
        SC.activation(std[:], mv[:, 1:2], AF.Sqrt, bias=eps_sb[:])
        thr = pstat.tile([128, 1], F32)
        V.tensor_tensor(thr[:], std[:], small["tp"][:], OP.mult)
        V.tensor_tensor(thr[:], thr[:], mv[:, 0:1], OP.add)
        osb = pstat.tile([128, TOK], BF16)
        V.tensor_scalar(osb[:], Yp[:], thr[:], None, OP.is_ge)
        nc.sync.dma_start(outT.ap(), osb[:])


def _tile_rows(a):
    # (8*128, N) -> (128, 8*N) so the SBUF [p, (t n)] load is contiguous
    n = a.shape[1]
    return np.ascontiguousarray(
        a.reshape(KT, 128, n).transpose(1, 0, 2).reshape(128, KT * n))


def _prep_inputs(inputs):
    x = np.asarray(inputs["x"], np.float32)
    xT = _tile_rows(x.reshape(TOK, D).T.astype(BF))
    Wg = np.asarray(inputs["Wg"], np.float64)
    wgr = (Wg.reshape(H, HD, H).sum(axis=1).T / 1024.0).astype(np.float32)
    wgr = np.ascontiguousarray(wgr)                     # [h, h']
    bgr = np.asarray(inputs["bg"], np.float32).reshape(H, 1)
    i2e = np.zeros((CH, 2), np.float32)
    i2e[0:HD, 0] = 1.0
    i2e[HD:CH, 1] = 1.0
    i16 = np.zeros((H, D), np.float32)
    for h in range(H):
        i16[h, h * HD:(h + 1) * HD] = 1.0
    i16 = np.ascontiguousarray(
        i16.reshape(H, KT, 128).reshape(H, KT * 128))
    in_maps = []
    for c in range(NCORES):
        sl = slice(CH * c, CH * c + CH)
        m = {"xT": xT, "wgr": wgr, "bgr": bgr, "i2e": i2e, "i16": i16}
        for nm in ("q", "k", "v", "p"):
            W = np.asarray(inputs[f"W{nm}"], np.float32)
            m["w" + nm] = _tile_rows(W[sl, :].T.astype(BF))
            g = np.asarray(inputs[f"g{nm}"], np.float32)[sl]
            be = np.asarray(inputs[f"beta{nm}"], np.float32)[sl]
            m["t" + nm] = ((2.0 - be) / g).reshape(CH, 1).astype(np.float32)
            m["b" + nm] = np.asarray(
                inputs[f"b{nm}"], np.float32)[sl].reshape(CH, 1)
        in_maps.append(m)
    return in_maps


def _run(inputs, trace=False):
    if "nc" not in _CACHE:
        _CACHE["nc"] = _build()
    nc = _CACHE["nc"]
    in_maps = _prep_inputs(inputs)
    res = run_bass_kernel_spmd(nc, in_maps, core_ids=list(range(NCORES)),
                               trace=trace)
    out = np.empty((TOK, D), np.float32)
    for c in range(NCORES):
        out[:, CH * c:CH * c + CH] = res.results[c]["outT"].astype(np.float32).T
    return out.reshape(B, NSEQ, D), res


def kernel(**inputs) -> np.ndarray:
    out, _ = _run(inputs, trace=False)
    return out


# revision 13
# speedup vs baseline: 1.6908x; 1.0126x over previous
"""Trainium2 Bass kernel for nn_LogicGatedSpikingSelfAttention.

Sharding: channel/head-parallel over 8 cores. Each core owns 128 output
channels = 2 heads for the q/k/v branches (BN stats fully local, since
stats are per-channel over all tokens), runs attention for its 2 heads
over all 4 batches locally, and computes a 128-output-channel slice of
the projection. One AllGather moves the binary attention spikes (+ per-
head energies for the logic gate) between the attention and projection
stages; the gate is folded into the projection weights after the gather
(exact: gate is {0,1}).

Attention uses associativity — there is no softmax, so
x_attn = scale*gate * q @ (k^T @ v). k^T@v is a 64x64 integer count
matrix per (batch, head) (exact in fp16, counts <= 1024), and
S = (k^T v)^T q gives channel-major integer scores identical to the
naive q@k^T@v order. The attn-LIF threshold reduces to S >= 2^0.75.
k/v spikes are transposed to token-major via the DMA XBAR (off the PE).
Per-head small matmuls are packed into PE quadrants via tile_position.
The spike payload crosses cores as fp8e4 ({0,1} exact), energies ride
along as bitcast f32 bytes.
"""
import numpy as np
import ml_dtypes

import concourse.bass as bass
import concourse.bacc as bacc
import concourse.tile as tile
from concourse import mybir
from concourse.bass_utils import run_bass_kernel_spmd

NCORES = 8
B, NSEQ, D, H = 4, 1024, 1024, 16
HD = D // H            # 64 head dim
CH = D // NCORES       # 128 channels per core
TOK = B * NSEQ         # 4096 tokens
KT = D // 128          # 8 contraction tiles
NBLK = TOK // 128      # 32 token blocks of 128
EPS = 1e-5
S_TH = float(2.0 ** 0.75)   # x_attn >= 1  <=>  S >= hd**0.125 = 2^0.75
SPIKE_N = 128 * TOK         # flat fp8 payload: spikes then 32B f32 energies
PAYLEN = SPIKE_N + 32
F32 = mybir.dt.float32
BF16 = mybir.dt.bfloat16
FP16 = mybir.dt.float16
FP8 = mybir.dt.float8e4
BF = ml_dtypes.bfloat16

_CACHE = {}


def _build():
    nc = bacc.Bacc("TRN2", target_bir_lowering=False, debug=False,
                   num_devices=NCORES)
    inp = {}
    def din(name, shape, dt=BF16):
        inp[name] = nc.dram_tensor(name, shape, dt, kind="ExternalInput")
        return inp[name]

    din("xT",  [128, KT * TOK])          # host pre-tiled: [p, (t n)]
    din("wq",  [128, KT * CH]); din("wk", [128, KT * CH])
    din("wv",  [128, KT * CH]); din("wp", [128, KT * CH])
    for nm in ("tq", "tk", "tv", "tp"):
        din(nm, [CH, 1], F32)
    din("wgr", [H, H], F32)              # lhsT: [h, h'] = sum_r Wg[h', h+16r]/1024
    din("bgr", [H, 1], F32)
    din("i2e", [CH, 2], F32)             # [p, j] = (p//64==j)
    din("i16", [H, KT * 128], F32)       # [h, (t m)] = (t*128+m)//64 == h
    outT = nc.dram_tensor("outT", [CH, TOK], BF16, kind="ExternalOutput")

    with tile.TileContext(nc) as tc:
        with tc.tile_pool(name="consts", bufs=1) as consts, \
             tc.tile_pool(name="spikes", bufs=1) as spk_pool, \
             tc.tile_pool(name="dram", bufs=1, space="DRAM") as dram:
            _body(tc, inp, outT, consts, spk_pool, dram)
    nc.compile()
    return nc


def _body(tc, inp, outT, consts, spk_pool, dram):
    nc = tc.nc
    V, SC, GP, TE = nc.vector, nc.scalar, nc.gpsimd, nc.tensor
    AF = mybir.ActivationFunctionType
    OP = mybir.AluOpType
    DENG = [nc.sync, nc.scalar, nc.gpsimd]

    # ---- constants / weights to SBUF (all host-contiguous) ----
    w_sb = {}
    for i, nm in enumerate(("wq", "wk", "wv", "wp")):
        t = consts.tile([128, KT, CH], BF16, name=f"{nm}_sb")
        DENG[i % 3].dma_start(
            t[:], inp[nm].ap().rearrange("p (t m) -> p t m", t=KT))
        w_sb[nm] = t
    small = {}
    for nm in ("tq", "tk", "tv", "tp", "bgr"):
        t = consts.tile([inp[nm].shape[0], 1], F32, name=f"{nm}_sb")
        nc.sync.dma_start(t[:], inp[nm].ap())
        small[nm] = t
    wgr_sb = consts.tile([H, H], F32)
    nc.sync.dma_start(wgr_sb[:], inp["wgr"].ap())
    i2e_sb = consts.tile([CH, 2], F32)
    nc.sync.dma_start(i2e_sb[:], inp["i2e"].ap())
    i16_sb = consts.tile([H, KT, 128], F32)
    nc.sync.dma_start(i16_sb[:],
                      inp["i16"].ap().rearrange("h (t m) -> h t m", t=KT))
    eps_sb = consts.tile([128, 1], F32)
    V.memset(eps_sb[:], EPS)

    # ---- persistent spike tensors ----
    spA = {nm: spk_pool.tile([128, TOK], FP16, name=f"sp{nm}A")
           for nm in ("q", "k", "v")}
    ktok = spk_pool.tile([128, NBLK, 128], FP16)   # [tok, blk, ch]
    vtok = spk_pool.tile([128, NBLK, 128], FP16)
    payload = spk_pool.tile([128, TOK], FP8)       # [64h+d, tok] spikes
    e_sb = spk_pool.tile([2, B], F32)

    # ================= branches (q, k, v) =================
    # Linear bias cancels inside BatchNorm (the mean absorbs it), so BN
    # stats and the spike threshold read directly from PSUM — no bias add
    # and no materialized Y.
    with tc.tile_pool(name="xts_p", bufs=1) as xts_p, \
         tc.tile_pool(name="stps", bufs=2) as stp:
        xre = inp["xT"].ap().rearrange("p (t n) -> p t n", t=KT)
        xts = []
        for kt in range(KT):
            t = xts_p.tile([128, TOK], BF16, name=f"xts{kt}")
            DENG[kt % 3].dma_start(t[:], xre[:, kt, :])
            xts.append(t)

        for nm in ("q", "k", "v"):
            # weight-stationary: kt outer, 8 PSUM banks accumulate
            with tc.tile_pool(name=f"brps_{nm}", bufs=1, space="PSUM") as brps:
                ps = [brps.tile([128, 512], F32, name=f"ps{nm}{i}")
                      for i in range(8)]
                for kt in range(KT):
                    for nck in range(8):
                        TE.matmul(ps[nck][:], w_sb["w" + nm][:, kt, :],
                                  xts[kt][:, nck * 512:(nck + 1) * 512],
                                  start=(kt == 0), stop=(kt == KT - 1))
                stats = stp.tile([128, 8, 6], F32, tag="stats")
                for i in range(8):
                    V.bn_stats(stats[:, i, :], ps[i][:])
                mv = stp.tile([128, 2], F32, tag="mv")
                V.bn_aggr(mv[:], stats[:])
                std = stp.tile([128, 1], F32, tag="std")
                SC.activation(std[:], mv[:, 1:2], AF.Sqrt, bias=eps_sb[:])
                thr = stp.tile([128, 1], F32, tag="thr")
                V.tensor_tensor(thr[:], std[:], small["t" + nm][:], OP.mult)
                V.tensor_tensor(thr[:], thr[:], mv[:, 0:1], OP.add)
                for b in range(B):
                    for j in range(2):
                        i = 2 * b + j
                        V.tensor_scalar(spA[nm][:, i * 512:(i + 1) * 512],
                                        ps[i][:], thr[:], None, OP.is_ge)
                    # token-major spikes for k/v via DMA XBAR (off the PE)
                    if nm == "k":
                        nc.sync.dma_start_transpose(
                            ktok[:, 8 * b:8 * b + 8, :],
                            spA["k"][:, b * NSEQ:(b + 1) * NSEQ])
                    elif nm == "v":
                        nc.scalar.dma_start_transpose(
                            vtok[:, 8 * b:8 * b + 8, :],
                            spA["v"][:, b * NSEQ:(b + 1) * NSEQ])

            if nm == "k":
                # energy elementwise part on gpsimd (overlaps v branch)
                prod = spk_pool.tile([128, TOK], FP16)
                GP.tensor_tensor(prod[:], spA["q"][:], spA["k"][:], OP.mult)
                ech = spk_pool.tile([128, B], F32)
                V.reduce_sum(ech[:],
                             prod[:].rearrange("p (b n) -> p b n", b=B),
                             axis=mybir.AxisListType.X)

    # ================= energy head-sum + attention =================
    with tc.tile_pool(name="atps", bufs=1, space="PSUM") as atps, \
         tc.tile_pool(name="s2ps", bufs=2, space="PSUM") as s2ps, \
         tc.tile_pool(name="kvsb", bufs=1) as kvsb:
        e_ps = atps.tile([2, B], F32, name="eps")
        TE.matmul(e_ps[:], i2e_sb[:], ech[:], start=True, stop=True)
        V.tensor_copy(e_sb[:], e_ps[:])

        # KV[b] = k_tok^T @ v_tok per head, heads packed in PE columns
        kv_ps = [atps.tile([128, HD], F32, name=f"kvps{b}") for b in range(B)]
        kv = kvsb.tile([128, B, HD], FP16)
        for b in range(B):
            for mt in range(8):
                blk = b * 8 + mt
                TE.matmul(kv_ps[b][0:HD, :], ktok[:, blk, 0:HD],
                          vtok[:, blk, 0:HD],
                          start=(mt == 0), stop=(mt == 7),
                          tile_position=(0, 0))
                TE.matmul(kv_ps[b][HD:128, :], ktok[:, blk, HD:128],
                          vtok[:, blk, HD:128],
                          start=(mt == 0), stop=(mt == 7),
                          tile_position=(0, HD))
            if b % 2:
                V.tensor_copy(kv[:, b, :], kv_ps[b][:])
            else:
                SC.activation(kv[:, b, :], kv_ps[b][:], AF.Copy)

        # S^T = KV^T @ q  (channel-major scores), heads packed in quadrants
        pay_d = dram.tile([PAYLEN], FP8)
        pay_re = pay_d[0:SPIKE_N].rearrange("(p n) -> p n", p=128)
        for b in range(B):
            for ncn in range(2):
                n0 = b * NSEQ + ncn * 512
                s2 = s2ps.tile([128, 512], F32, tag="s2")
                TE.matmul(s2[0:HD, :], kv[0:HD, b, :],
                          spA["q"][0:HD, n0:n0 + 512],
                          start=True, stop=True, tile_position=(0, 0))
                TE.matmul(s2[HD:128, :], kv[HD:128, b, :],
                          spA["q"][HD:128, n0:n0 + 512],
                          start=True, stop=True, tile_position=(HD, HD))
                V.tensor_scalar(payload[:, n0:n0 + 512], s2[:], S_TH,
                                None, OP.is_ge)
            DENG[b % 2].dma_start(
                pay_re[:, b * NSEQ:(b + 1) * NSEQ],
                payload[:, b * NSEQ:(b + 1) * NSEQ])

    # ================= AllGather (flat fp8, contiguous) =================
    gath_d = dram.tile([NCORES, PAYLEN], FP8, addr_space="Shared")
    nc.gpsimd.dma_start(
        pay_d[SPIKE_N:PAYLEN].rearrange("(p w) -> p w", p=2),
        e_sb[:].bitcast(FP8))
    GP.collective_compute("AllGather", OP.bypass,
                          ins=[pay_d.opt()], outs=[gath_d.opt()],
                          replica_groups=[list(range(NCORES))])

    # ================= gate -> gated proj weights =================
    with tc.tile_pool(name="gtmp", bufs=1) as gtmp, \
         tc.tile_pool(name="post", bufs=1) as post, \
         tc.tile_pool(name="rhsp", bufs=3) as rhsp, \
         tc.tile_pool(name="pstat", bufs=1) as pstat:
        with tc.tile_pool(name="gtps", bufs=2, space="PSUM") as gtps:
            eg_bytes = gtmp.tile([H, 16], FP8)
            for c in range(NCORES):
                DENG[c % 3].dma_start(
                    eg_bytes[2 * c:2 * c + 2, :],
                    gath_d[c, SPIKE_N:PAYLEN].rearrange("(p w) -> p w", p=2))
            g_ps = gtps.tile([H, B], F32, tag="gps")
            TE.matmul(g_ps[:], wgr_sb[:], eg_bytes[:].bitcast(F32),
                      start=True, stop=True)
            gate = gtmp.tile([H, B], F32)
            V.tensor_scalar(gate[:], g_ps[:], small["bgr"][:], 0.5,
                            OP.add, OP.is_ge)
            gv = gtmp.tile([128, KT, B], F32)
            for t in range(KT):
                gv_ps = gtps.tile([128, B], F32, tag="gvps")
                TE.matmul(gv_ps[:], i16_sb[:, t, :], gate[:],
                          start=True, stop=True)
                V.tensor_copy(gv[:, t, :], gv_ps[:])
            wpg = []
            for t in range(KT):
                w = post.tile([128, B, 128], BF16, name=f"wpg{t}")
                for b in range(B):
                    if (t * B + b) % 2:
                        V.tensor_scalar(w[:, b, :], w_sb["wp"][:, t, :],
                                        gv[:, t, b:b + 1], None, OP.mult)
                    else:
                        SC.activation(w[:, b, :], w_sb["wp"][:, t, :],
                                      AF.Identity, scale=gv[:, t, b:b + 1])
                wpg.append(w)

        # ================= projection (fp8 rhs, bf16 weights) ==========
        with tc.tile_pool(name="ppps", bufs=1, space="PSUM") as ppps:
            pp = [ppps.tile([128, 512], F32, name=f"pp{i}") for i in range(8)]
            rhs = []
            for t in range(KT):
                r = rhsp.tile([128, TOK], FP8, tag="rhs")
                DENG[t % 3].dma_start(
                    r[:],
                    gath_d[t, 0:SPIKE_N].rearrange("(p n) -> p n", p=128))
                rhs.append(r)
            for t in range(KT):
                for b in range(B):
                    for ncn in range(2):
                        n0 = b * NSEQ + ncn * 512
                        TE.matmul(pp[b * 2 + ncn][:], wpg[t][:, b, :],
                                  rhs[t][:, n0:n0 + 512],
                                  start=(t == 0), stop=(t == KT - 1))
            # BN stats + spike threshold directly from PSUM (bias cancels)
            stats = pstat.tile([128, 8, 6], F32)
            for i in range(8):
                V.bn_stats(stats[:, i, :], pp[i][:])
            mv = pstat.tile([128, 2], F32)
            V.bn_aggr(mv[:], stats[:])
            std = pstat.tile([128, 1], F32)
            SC.activation(std[:], mv[:, 1:2], AF.Sqrt, bias=eps_sb[:])
            thr = pstat.tile([128, 1], F32)
            V.tensor_tensor(thr[:], std[:], small["tp"][:], OP.mult)
            V.tensor_tensor(thr[:], thr[:], mv[:, 0:1], OP.add)
            osb = pstat.tile([128, TOK], BF16)
            for i in range(8):
                V.tensor_scalar(osb[:, i * 512:(i + 1) * 512], pp[i][:],
                                thr[:], None, OP.is_ge)
                DENG[i % 2].dma_start(
                    outT.ap().rearrange("p (c n) -> p c n", c=8)[:, i, :],
                    osb[:, i * 512:(i + 1) * 512])


def _tile_rows(a):
    # (8*128, N) -> (128, 8*N) so the SBUF [p, (t n)] load is contiguous
    n = a.shape[1]
    return np.ascontiguousarray(
        a.reshape(KT, 128, n).transpose(1, 0, 2).reshape(128, KT * n))


def _prep_inputs(inputs):
    x = np.asarray(inputs["x"], np.float32)
    xT = _tile_rows(x.reshape(TOK, D).T.astype(BF))
    Wg = np.asarray(inputs["Wg"], np.float64)
    wgr = (Wg.reshape(H, HD, H).sum(axis=1).T / 1024.0).astype(np.float32)
    wgr = np.ascontiguousarray(wgr)                     # [h, h']
    bgr = np.asarray(inputs["bg"], np.float32).reshape(H, 1)
    i2e = np.zeros((CH, 2), np.float32)
    i2e[0:HD, 0] = 1.0
    i2e[HD:CH, 1] = 1.0
    i16 = np.zeros((H, D), np.float32)
    for h in range(H):
        i16[h, h * HD:(h + 1) * HD] = 1.0
    i16 = np.ascontiguousarray(
        i16.reshape(H, KT, 128).reshape(H, KT * 128))
    in_maps = []
    for c in range(NCORES):
        sl = slice(CH * c, CH * c + CH)
        m = {"xT": xT, "wgr": wgr, "bgr": bgr, "i2e": i2e, "i16": i16}
        for nm in ("q", "k", "v", "p"):
            W = np.asarray(inputs[f"W{nm}"], np.float32)
            m["w" + nm] = _tile_rows(W[sl, :].T.astype(BF))
            g = np.asarray(inputs[f"g{nm}"], np.float32)[sl]
            be = np.asarray(inputs[f"beta{nm}"], np.float32)[sl]
            m["t" + nm] = ((2.0 - be) / g).reshape(CH, 1).astype(np.float32)
        in_maps.append(m)
    return in_maps


def _run(inputs, trace=False):
    if "nc" not in _CACHE:
        _CACHE["nc"] = _build()
    nc = _CACHE["nc"]
    in_maps = _prep_inputs(inputs)
    res = run_bass_kernel_spmd(nc, in_maps, core_ids=list(range(NCORES)),
                               trace=trace)
    out = np.empty((TOK, D), np.float32)
    for c in range(NCORES):
        out[:, CH * c:CH * c + CH] = res.results[c]["outT"].astype(np.float32).T
    return out.reshape(B, NSEQ, D), res


def kernel(**inputs) -> np.ndarray:
    out, _ = _run(inputs, trace=False)
    return out
